# revision 67
# baseline (speedup 1.0000x reference)
"""Multi-head attention (B=4, S=2048, D=1024, H=16) on 8 Trainium2 cores.

Sharding: core c handles batch b = c//2 and query-half qh = c%2 (1024 query
tokens). Each core computes full K/V projections for its batch (duplicated
across the 2 cores sharing a batch) so no cross-core collectives are needed.

v2 structure (ACT-overlapped): the softmax exp stream on the Scalar engine
(256 x [128,1024] tiles ~ 285us) is the pacing floor, so all projection
matmuls that the first attention pass doesn't need are deferred into the
attention phase where the Tile scheduler slots them into PE gaps while the
PE waits on exp:
  - prefix computes only K^T/Q^T for head-pairs 0 and 1
  - pass hp (1..6) emits K^T/Q^T projection for head-pair hp+1 AFTER its
    attention work, so those matmuls get popped only when scores/attnV are
    blocked on the exp
  - V projection (per k-chunk) is emitted inside pass 0's first q-half loop
    just ahead of the attnV that consumes it
  - out-projection stays a serial tail (needs every head's output)
Scores for both heads of a pair land in one [128,1024] PSUM tile (q split
into 512-halves) so each iteration runs ONE exp at FD=1024. ACT does
nothing but exp; bias adds and PSUM evacuations ride the Vector engine
(tensor_scalar with a [128,1] bias column). PSUM budget: scores 2x2 banks +
attnV accumulators 2x1 + scratch (proj/psb) 2x1 = 8 banks.

Layout strategy (all matmuls contract over the partition dim):
  - host ships x^T (d-major) so projections need no on-device transposes
  - Q^T, K^T produced as [dout(part), tok(free)]; scores^T = K_h^T.T @ Q_h^T
    -> [k(part), q(free)] with the two heads of a pair on partition halves
    0:64 / 64:128 (concurrent row-group matmuls)
  - V produced as [tok(part), dout(free)] with a ones column per head so
    attn@V also yields the softmax denominators (row 64 of the accumulator)
  - normalize via reciprocal + rank-1 ones x recip broadcast matmul; the
    out-proj consumes O^T tiles and writes [q(part), dout] straight to DRAM.
    bv/bo folded into a host-computed constant row added at the end.
"""
import sys

if "/opt/trn_rl_repo" not in sys.path:
    sys.path.insert(0, "/opt/trn_rl_repo")

import numpy as np
import ml_dtypes

import concourse.bacc as bacc
import concourse.mybir as mybir
from concourse.tile import TileContext
from concourse.bass_utils import run_bass_kernel_spmd

B, S, D, H = 4, 2048, 1024, 16
DH = D // H            # 64
QT = S // 2            # 1024 query tokens per core
QH = 512               # q half processed per attention pass
N_CORES = 8
PCH = D // 128         # 8 partition chunks of the model dim
KCH = S // 128         # 16 key-token chunks
VW = DH + 1            # 65: per-head V width incl. ones column
VPAD = H * VW + 63     # V tile width padded so a 128-col lhsT read never overruns

F32 = mybir.dt.float32
MM_DT = mybir.dt.bfloat16
NP_MM = ml_dtypes.bfloat16
FP8 = mybir.dt.float8e4
NP_FP8 = ml_dtypes.float8_e4m3
DR = mybir.MatmulPerfMode.DoubleRow

# fp8 scale factors: Wv/Wo ship as 32x so their sigma~0.64 uses the e4m3
# grid; O^T stores 16x (via the ones_t broadcast); the out-projection PSUM
# is 16*32 = 512x and gets rescaled during evacuation.
WV_SCALE = 32.0
OT_SCALE = 16.0
WO_SCALE = 32.0

AF = mybir.ActivationFunctionType
OP = mybir.AluOpType

DEBUG = False


def _emit(nc, tc):
    xqT = nc.dram_tensor("xqT", [D, QT], MM_DT, kind="ExternalInput")
    xkT = nc.dram_tensor("xkT", [D, S], MM_DT, kind="ExternalInput")
    xvT = nc.dram_tensor("xvT", [D, S], MM_DT, kind="ExternalInput")
    Wq = nc.dram_tensor("Wq", [D, D], MM_DT, kind="ExternalInput")
    Wk = nc.dram_tensor("Wk", [D, D], MM_DT, kind="ExternalInput")
    Wv = nc.dram_tensor("Wv", [D, D], MM_DT, kind="ExternalInput")
    Wo = nc.dram_tensor("Wo", [D, D], MM_DT, kind="ExternalInput")
    bqc = nc.dram_tensor("bqc", [128, PCH], F32, kind="ExternalInput")
    bkc = nc.dram_tensor("bkc", [128, PCH], F32, kind="ExternalInput")
    cbc = nc.dram_tensor("cbc", [128, D], F32, kind="ExternalInput")
    out = nc.dram_tensor("out", [QT, D], MM_DT, kind="ExternalOutput")

    # xvT viewed as [128, PCH, S] so one DMA fetches a [128, PCH*128]
    # column-block (all 8 d-chunks of one key-token chunk).
    xvT3 = xvT.rearrange("(c p) s -> p c s", p=128)

    dbg = {}
    if DEBUG:
        dbg["kt"] = nc.dram_tensor("dbg_kt", [PCH, 128, S], MM_DT, kind="ExternalOutput")
        dbg["qt"] = nc.dram_tensor("dbg_qt", [PCH, 128, QT], MM_DT, kind="ExternalOutput")
        dbg["v"] = nc.dram_tensor("dbg_v", [KCH, 128, VPAD], MM_DT, kind="ExternalOutput")
        dbg["ot"] = nc.dram_tensor("dbg_ot", [PCH, 128, QT], MM_DT, kind="ExternalOutput")
        dbg["pt"] = nc.dram_tensor("dbg_pt", [2, 128, 2 * QH], MM_DT, kind="ExternalOutput")
        dbg["rc"] = nc.dram_tensor("dbg_rc", [2, 1, QH], F32, kind="ExternalOutput")

    from contextlib import ExitStack
    with ExitStack() as stack:
        pool = lambda name, bufs, **kw: stack.enter_context(
            tc.tile_pool(name=name, bufs=bufs, **kw))
        xkp = pool("xkp", PCH)            # xk chunks, live to hp6
        wkp = pool("wkp", PCH)
        xqp = pool("xqp", PCH)
        wqp = pool("wqp", PCH)
        xvp = pool("xvp", 2)              # [128, 1024] column-blocks
        wp = pool("wp", PCH)              # Wv then Wo
        ktp = pool("ktp", 3)              # K^T rolling
        qtp = pool("qtp", 3)              # Q^T rolling
        vp = pool("vp", KCH)              # V (ones-augmented) resident
        otp = pool("otp", PCH)            # O^T resident
        misc = pool("misc", 1)
        ptp = pool("ptp", 5)              # P^T staging
        rcp = pool("rcp", 2)
        bbp = pool("bbp", 2)
        outp = pool("outp", 3)
        bq_t = misc.tile([128, PCH], F32, name="bq_t")
        nc.sync.dma_start(out=bq_t[:, :], in_=bqc[:, :])
        bk_t = misc.tile([128, PCH], F32, name="bk_t")
        nc.sync.dma_start(out=bk_t[:, :], in_=bkc[:, :])
        cb_t = misc.tile([128, D], F32, name="cb_t")
        nc.sync.dma_start(out=cb_t[:, :], in_=cbc[:, :])
        ones_f = misc.tile([1, DH], F32, name="ones_f")
        nc.vector.memset(ones_f[:, :], 1.0)
        ones_t = misc.tile([1, DH], mybir.dt.float32r, name="ones_t")
        nc.vector.tensor_copy(ones_t[:, :], ones_f[:, :])

        # ---- prefix DMAs in three sequenced batches (K inputs get the HBM
        # bandwidth first; Q's batch starts only once K's data has landed,
        # then V's) so the first projections aren't starved by round-robin
        # across all queues.
        from concourse.tile_rust import add_dep_helper
        # DMA order: the K0-nb0/nb1 + Q0 critical set (xk halves 0, wk0, xq,
        # wq0) drains first; then xk halves 1 (K0-nb2/3, woven into pass 0),
        # then V's inputs, then the remaining weight chunks for the weave.
        xk_t = [xkp.tile([128, S], MM_DT, name=f"xk{i}", tag="xk")
                for i in range(PCH)]
        for i in range(PCH):
            nc.sync.dma_start(out=xk_t[i][:, 0:1024],
                              in_=xkT[i * 128:(i + 1) * 128, 0:1024])
        wk_t = [wkp.tile([128, D], MM_DT, name=f"wk{i}", tag="wk")
                for i in range(PCH)]
        nc.sync.dma_start(out=wk_t[0][:, :], in_=Wk[0:128, :])
        xq_t = []
        for i in range(PCH):
            xq = xqp.tile([128, QT], MM_DT, name=f"xq{i}", tag="xq")
            nc.sync.dma_start(out=xq[:, :], in_=xqT[i * 128:(i + 1) * 128, :])
            xq_t.append(xq)
        wq_t = [wqp.tile([128, D], MM_DT, name=f"wq{i}", tag="wq")
                for i in range(PCH)]
        nc.sync.dma_start(out=wq_t[0][:, :], in_=Wq[0:128, :])
        for i in range(PCH):
            nc.sync.dma_start(out=xk_t[i][:, 1024:2048],
                              in_=xkT[i * 128:(i + 1) * 128, 1024:2048])
        wv_t = []
        for i in range(PCH):
            wv = wp.tile([128, D], MM_DT, name=f"wv{i}", tag="w")
            nc.sync.dma_start(out=wv[:, :], in_=Wv[i * 128:(i + 1) * 128, :])
            wv_t.append(wv)
        xv_c = [xvp.tile([128, PCH * 128], MM_DT, name=f"xv{t}", tag="xv")
                for t in range(KCH)]

        def dma_xv(t):
            nc.sync.dma_start(
                out=xv_c[t][:, :].rearrange("p (c s) -> p c s", c=PCH),
                in_=xvT3[:, :, t * 128:(t + 1) * 128],
            )

        for t in range(KCH):
            dma_xv(t)
        for i in range(1, PCH):
            nc.sync.dma_start(out=wk_t[i][:, :], in_=Wk[i * 128:(i + 1) * 128, :])
            nc.sync.dma_start(out=wq_t[i][:, :], in_=Wq[i * 128:(i + 1) * 128, :])

        psum_stack = stack.enter_context(ExitStack())
        ppool = lambda name, bufs: psum_stack.enter_context(
            tc.tile_pool(name=name, bufs=bufs, space="PSUM"))
        pssp = ppool("pss", 2)
        pop = ppool("pop", 2)
        scrp = ppool("scr", 2)
        if True:
            kt_t, qt_t = {}, {}

            def proj_steps(which, m):
                """Emit-closures for one K^T/Q^T projection, one matmul per
                step so they can be woven between attention iterations."""
                if which == "k":
                    W, X, bias, nnb, dst = wk_t, xk_t, bk_t, S // 512, \
                        ktp.tile([128, S], MM_DT, name=f"kt{m}", tag="kt")
                    kt_t[m] = dst
                else:
                    W, X, bias, nnb, dst = wq_t, xq_t, bq_t, QT // 512, \
                        qtp.tile([128, QT], MM_DT, name=f"qt{m}", tag="qt")
                    qt_t[m] = dst
                steps = []
                for nb in range(nnb):
                    box = {}
                    def mk(nb, kk, box):
                        def step():
                            if kk == 0:
                                box["ps"] = scrp.tile(
                                    [128, 512], F32,
                                    name=f"ps{which}{m}_{nb}", tag="scr")
                            nc.tensor.matmul(
                                box["ps"][:, :],
                                lhsT=W[kk][:, m * 128:(m + 1) * 128],
                                rhs=X[kk][:, nb * 512:(nb + 1) * 512],
                                start=(kk == 0), stop=(kk == PCH - 1),
                                skip_group_check=True,
                            )
                            if kk == PCH - 1:
                                nc.vector.tensor_scalar(
                                    dst[:, nb * 512:(nb + 1) * 512],
                                    box["ps"][:, :],
                                    bias[:, m:m + 1], None, OP.add,
                                )
                                if DEBUG and nb == nnb - 1:
                                    nc.sync.dma_start(
                                        out=dbg["kt" if which == "k" else "qt"][m],
                                        in_=dst[:, :])
                        return step
                    for kk in range(PCH):
                        steps.append(mk(nb, kk, box))
                return steps

            def kproj(m):
                for s in proj_steps("k", m):
                    s()

            def qproj(m):
                for s in proj_steps("q", m):
                    s()

            v_t = [vp.tile([128, VPAD], MM_DT, name=f"v{t}", tag="v")
                   for t in range(KCH)]

            def vproj(t):
                oc = v_t[t][:, 0:H * VW].rearrange("p (h x) -> p h x", x=VW)
                nc.vector.memset(oc[:, :, DH:VW], 1.0)
                nc.vector.memset(v_t[t][:, H * VW:VPAD], 0.0)
                for db in range(D // 512):
                    ps = scrp.tile([128, 512], F32, name=f"psv{t}_{db}", tag="scr")
                    for kk in range(PCH):
                        nc.tensor.matmul(
                            ps[:, :],
                            lhsT=xv_c[t][:, kk * 128:(kk + 1) * 128],
                            rhs=wv_t[kk][:, db * 512:(db + 1) * 512],
                            start=(kk == 0), stop=(kk == PCH - 1),
                        )
                    dst = oc[:, db * 8:(db + 1) * 8, 0:DH]
                    src = ps[:, :].rearrange("p (h d) -> p h d", d=DH)
                    nc.vector.tensor_copy(dst, src)
                if DEBUG:
                    nc.sync.dma_start(out=dbg["v"][t], in_=v_t[t][:, :])

            # prefix: K0's first two token-blocks + all of Q0 (covers pass-0
            # scores t<8); K0-nb2/3 weave into pass 0 once xk's second halves
            # land. Pair 1's projections weave into pass 0's second q-half.
            k0_steps = proj_steps("k", 0)
            for s in k0_steps[0:16]:
                s()
            qproj(0)
            k0_rest = k0_steps[16:]

            ot_t = [otp.tile([128, QT], MM_DT, name=f"ot{i}", tag="ot")
                    for i in range(PCH)]

            def attn_v(hp, t, po, pt):
                # lhsT reads 128 cols (overlapping the next head's V block) so
                # the weight load takes the fast path; PSUM rows 65-127 get
                # garbage that is never read.
                for j in range(2):
                    h = 2 * hp + j
                    nc.tensor.matmul(
                        po[j][:, :],
                        lhsT=v_t[t][:, h * VW:h * VW + 128],
                        rhs=pt[:, j * QH:(j + 1) * QH],
                        start=(t == 0), stop=(t == KCH - 1),
                        skip_group_check=True,
                    )

            # Boundary work deferred into the following pass so it never
            # head-of-line blocks the PE stream: pending_tail holds the last
            # attnV + PSUM-evacuating copies (phase A), norm_b the reciprocal
            # -> broadcast -> multiply chain (phase B).
            norm_b = []
            pending_tail = []

            for hp in range(H // 2):
                # deferred projections for head-pair hp+1, woven one matmul
                # at a time into this pass's iteration stream so they fill
                # the PE slack while the exp stream paces the pass. Pass 0
                # weaves only in its second q-half (the weights arrive late
                # and a stalled weave matmul would block the whole stream).
                weave = []
                if hp <= H // 2 - 2:
                    weave = proj_steps("k", hp + 1) + proj_steps("q", hp + 1)
                wi = 0
                for qh in range(2):
                    po = [pop.tile([128, QH], F32, name=f"po{hp}_{qh}_{j}", tag="po")
                          for j in range(2)]
                    pt_prev = None
                    for t in range(KCH):
                        pss = pssp.tile([128, 2 * QH], F32,
                                        name=f"pss{hp}_{qh}_{t}", tag="pss")
                        for j in range(2):
                            lo, hi = j * 64, (j + 1) * 64
                            nc.tensor.matmul(
                                pss[:, j * QH:(j + 1) * QH],
                                lhsT=kt_t[hp][lo:hi, t * 128:(t + 1) * 128],
                                rhs=qt_t[hp][lo:hi, qh * QH:(qh + 1) * QH],
                                start=True, stop=True,
                            )
                        pt = ptp.tile([128, 2 * QH], MM_DT,
                                      name=f"pt{hp}_{qh}_{t}", tag="pt")
                        nc.scalar.activation(pt[:, :], pss[:, :], AF.Exp,
                                             scale=1.0 / 8.0)
                        if DEBUG and hp == 0 and qh == 0 and t < 2:
                            nc.sync.dma_start(out=dbg["pt"][t], in_=pt[:, :])
                        if t == 1 and pending_tail:
                            for fn in pending_tail:
                                fn()
                            pending_tail = []
                        if t == 2 and norm_b:
                            for fn in norm_b:
                                fn()
                            norm_b = []
                        if hp == 0 and qh == 0:
                            if 5 <= t <= 8:
                                for s in k0_rest[(t - 5) * 4:(t - 4) * 4]:
                                    s()
                            vproj(t)
                        if hp > 0 or qh == 1:
                            # finish the weave ~4 iterations before pass end
                            # so kt[hp+1]'s final evacuation never gates the
                            # next pass's first scores.
                            done = (qh * KCH + t) if hp else t
                            it_left = (2 * KCH if hp else KCH) - 4 - done
                            n_pop = (len(weave) - wi + it_left - 1) // it_left \
                                if it_left > 0 else len(weave) - wi
                            for _ in range(n_pop):
                                if wi < len(weave):
                                    weave[wi]()
                                    wi += 1
                        if pt_prev is not None:
                            attn_v(hp, t - 1, po, pt_prev)
                        pt_prev = pt

                    def mk_tail_a(hp, qh, po, pt_last):
                        def tail():
                            attn_v(hp, KCH - 1, po, pt_last)
                            # phase A: copy sums row + O rows off PSUM so the
                            # po accumulators recycle.
                            sums_j, ou_j = [], []
                            for j in range(2):
                                sums = rcp.tile([1, QH], F32,
                                                name=f"sm{hp}_{qh}_{j}", tag="sm")
                                nc.vector.tensor_copy(sums[:, :], po[j][64:65, :])
                                ou = bbp.tile([64, QH], F32,
                                              name=f"ou{hp}_{qh}_{j}", tag="ou")
                                nc.vector.tensor_copy(ou[:, :], po[j][0:64, :])
                                sums_j.append(sums)
                                ou_j.append(ou)
                            norm_b.append(mk_norm_b(hp, qh, sums_j, ou_j))
                        return tail

                    pending_tail.append(mk_tail_a(hp, qh, po, pt_prev))

                    def mk_norm_b(hp, qh, sums_j, ou_j):
                        def norm():
                            for j in range(2):
                                recip_f = rcp.tile([1, QH], F32,
                                                   name=f"rf{hp}_{qh}_{j}", tag="rf")
                                nc.vector.reciprocal_approx_fast(
                                    recip_f[:, :], sums_j[j][:, :])
                                recip = rcp.tile([1, QH], mybir.dt.float32r,
                                                 name=f"rc{hp}_{qh}_{j}", tag="rc")
                                nc.vector.tensor_copy(recip[:, :], recip_f[:, :])
                                if DEBUG and hp == 0 and qh == 0:
                                    nc.sync.dma_start(out=dbg["rc"][j],
                                                      in_=recip_f[:, :])
                                psb = scrp.tile([128, QH], F32,
                                                name=f"psb{hp}_{qh}_{j}", tag="scr")
                                nc.tensor.matmul(
                                    psb[0:64, :], lhsT=ones_t[:, :],
                                    rhs=recip[:, :], start=True, stop=True,
                                )
                                nc.vector.tensor_tensor(
                                    ot_t[hp][j * 64:(j + 1) * 64,
                                             qh * QH:(qh + 1) * QH],
                                    ou_j[j][:, :], psb[0:64, :], OP.mult,
                                )
                        return norm

                while wi < len(weave):
                    weave[wi]()
                    wi += 1

            for fn in pending_tail:
                fn()
            for fn in norm_b:
                fn()

        # ---- out-proj tail: out = O^T.T @ Wo + (bv@Wo + bo) ---------------
        psum_stack.close()
        wo_t = []
        for i in range(PCH):
            wo = wp.tile([128, D], MM_DT, name=f"wo{i}", tag="w")
            nc.sync.dma_start(out=wo[:, :], in_=Wo[i * 128:(i + 1) * 128, :])
            wo_t.append(wo)
        with tc.tile_pool(name="ps3", bufs=3, space="PSUM") as ps3:
            for qc in range(QT // 128):
                for db in range(D // 512):
                    ps = ps3.tile([128, 512], F32, name=f"pso{qc}_{db}", tag="ps3")
                    for vc in range(PCH):
                        nc.tensor.matmul(
                            ps[:, :],
                            lhsT=ot_t[vc][:, qc * 128:(qc + 1) * 128],
                            rhs=wo_t[vc][:, db * 512:(db + 1) * 512],
                            start=(vc == 0), stop=(vc == PCH - 1),
                        )
                    osb = outp.tile([128, 512], MM_DT, name=f"osb{qc}_{db}", tag="osb")
                    nc.vector.tensor_tensor(osb[:, :], ps[:, :],
                                            cb_t[:, db * 512:(db + 1) * 512], OP.add)
                    nc.sync.dma_start(
                        out=out[qc * 128:(qc + 1) * 128, db * 512:(db + 1) * 512],
                        in_=osb[:, :],
                    )


_NC_CACHE = None


def build_nc():
    global _NC_CACHE
    if _NC_CACHE is None:
        nc = bacc.Bacc("TRN2", target_bir_lowering=False, debug=False,
                       num_devices=N_CORES)
        with TileContext(nc) as tc:
            _emit(nc, tc)
        nc.compile()
        _NC_CACHE = nc
    return _NC_CACHE


def make_in_maps(query, key, value, Wq, bq, Wk, bk, Wv, bv, Wo, bo):
    c = (bv.astype(np.float32) @ Wo.astype(np.float32)) + bo.astype(np.float32)

    def q8(x, scale):
        return np.clip(np.asarray(x, np.float32) * scale, -240.0, 240.0).astype(NP_FP8)

    shared = {
        "Wq": np.ascontiguousarray(Wq, dtype=NP_MM),
        "Wk": np.ascontiguousarray(Wk, dtype=NP_MM),
        "Wv": np.ascontiguousarray(Wv, dtype=NP_MM),
        "Wo": np.ascontiguousarray(Wo, dtype=NP_MM),
        "bqc": np.ascontiguousarray(bq.reshape(PCH, 128).T, dtype=np.float32),
        "bkc": np.ascontiguousarray(bk.reshape(PCH, 128).T, dtype=np.float32),
        "cbc": np.ascontiguousarray(np.broadcast_to(c, (128, D)), dtype=np.float32),
    }
    in_maps = []
    for core in range(N_CORES):
        b, qh = core // 2, core % 2
        in_maps.append(dict(
            shared,
            xqT=np.ascontiguousarray(query[b, qh * QT:(qh + 1) * QT, :].T, dtype=NP_MM),
            xkT=np.ascontiguousarray(key[b].T, dtype=NP_MM),
            xvT=np.ascontiguousarray(value[b].T, dtype=NP_MM),
        ))
    return in_maps


def run(in_maps, trace=False):
    nc = build_nc()
    return run_bass_kernel_spmd(nc, in_maps, list(range(N_CORES)), trace=trace)


def kernel(query, key, value, mask, Wq, bq, Wk, bk, Wv, bv, Wo, bo):
    query = np.asarray(query, dtype=np.float32)
    key = np.asarray(key, dtype=np.float32)
    value = np.asarray(value, dtype=np.float32)
    # mask is all-ones by construction (spec fill: ones) — no-op in the math.
    in_maps = make_in_maps(query, key, value,
                           np.asarray(Wq), np.asarray(bq), np.asarray(Wk),
                           np.asarray(bk), np.asarray(Wv), np.asarray(bv),
                           np.asarray(Wo), np.asarray(bo))
    res = run(in_maps, trace=False)
    out = np.empty((B, S, D), np.float32)
    for core in range(N_CORES):
        b, qh = core // 2, core % 2
        out[b, qh * QT:(qh + 1) * QT, :] = np.asarray(
            res.results[core]["out"], dtype=np.float32)
    return out


# revision 68
# speedup vs baseline: 1.0327x; 1.0327x over previous
"""Multi-head attention (B=4, S=2048, D=1024, H=16) on 8 Trainium2 cores.

Sharding: core c handles batch b = c//2 and query-half qh = c%2 (1024 query
tokens). Each core computes full K/V projections for its batch (duplicated
across the 2 cores sharing a batch) so no cross-core collectives are needed.

v2 structure (ACT-overlapped): the softmax exp stream on the Scalar engine
(256 x [128,1024] tiles ~ 285us) is the pacing floor, so all projection
matmuls that the first attention pass doesn't need are deferred into the
attention phase where the Tile scheduler slots them into PE gaps while the
PE waits on exp:
  - prefix computes only K^T/Q^T for head-pairs 0 and 1
  - pass hp (1..6) emits K^T/Q^T projection for head-pair hp+1 AFTER its
    attention work, so those matmuls get popped only when scores/attnV are
    blocked on the exp
  - V projection (per k-chunk) is emitted inside pass 0's first q-half loop
    just ahead of the attnV that consumes it
  - out-projection stays a serial tail (needs every head's output)
Scores for both heads of a pair land in one [128,1024] PSUM tile (q split
into 512-halves) so each iteration runs ONE exp at FD=1024. ACT does
nothing but exp; bias adds and PSUM evacuations ride the Vector engine
(tensor_scalar with a [128,1] bias column). PSUM budget: scores 2x2 banks +
attnV accumulators 2x1 + scratch (proj/psb) 2x1 = 8 banks.

Layout strategy (all matmuls contract over the partition dim):
  - host ships x^T (d-major) so projections need no on-device transposes
  - Q^T, K^T produced as [dout(part), tok(free)]; scores^T = K_h^T.T @ Q_h^T
    -> [k(part), q(free)] with the two heads of a pair on partition halves
    0:64 / 64:128 (concurrent row-group matmuls)
  - V produced as [tok(part), dout(free)] with a ones column per head so
    attn@V also yields the softmax denominators (row 64 of the accumulator)
  - normalize via reciprocal + rank-1 ones x recip broadcast matmul; the
    out-proj consumes O^T tiles and writes [q(part), dout] straight to DRAM.
    bv/bo folded into a host-computed constant row added at the end.
"""
import sys

if "/opt/trn_rl_repo" not in sys.path:
    sys.path.insert(0, "/opt/trn_rl_repo")

import numpy as np
import ml_dtypes

import concourse.bacc as bacc
import concourse.mybir as mybir
from concourse.tile import TileContext
from concourse.bass_utils import run_bass_kernel_spmd

B, S, D, H = 4, 2048, 1024, 16
DH = D // H            # 64
QT = S // 2            # 1024 query tokens per core
QH = 512               # q half processed per attention pass
N_CORES = 8
PCH = D // 128         # 8 partition chunks of the model dim
KCH = S // 128         # 16 key-token chunks
VW = DH + 1            # 65: per-head V width incl. ones column
VPAD = H * VW + 63     # V tile width padded so a 128-col lhsT read never overruns

F32 = mybir.dt.float32
MM_DT = mybir.dt.bfloat16
NP_MM = ml_dtypes.bfloat16
FP8 = mybir.dt.float8e4
NP_FP8 = ml_dtypes.float8_e4m3
DR = mybir.MatmulPerfMode.DoubleRow

# fp8 scale factors: Wv/Wo ship as 32x so their sigma~0.64 uses the e4m3
# grid; O^T stores 16x (via the ones_t broadcast); the out-projection PSUM
# is 16*32 = 512x and gets rescaled during evacuation.
WV_SCALE = 32.0
OT_SCALE = 16.0
WO_SCALE = 32.0

AF = mybir.ActivationFunctionType
OP = mybir.AluOpType

DEBUG = False


def _emit(nc, tc):
    xqT = nc.dram_tensor("xqT", [D, QT], MM_DT, kind="ExternalInput")
    xkT = nc.dram_tensor("xkT", [D, S], MM_DT, kind="ExternalInput")
    xvT = nc.dram_tensor("xvT", [D, S], MM_DT, kind="ExternalInput")
    Wq = nc.dram_tensor("Wq", [D, D], MM_DT, kind="ExternalInput")
    Wk = nc.dram_tensor("Wk", [D, D], MM_DT, kind="ExternalInput")
    Wv = nc.dram_tensor("Wv", [D, D], MM_DT, kind="ExternalInput")
    Wo = nc.dram_tensor("Wo", [D, D], MM_DT, kind="ExternalInput")
    bqc = nc.dram_tensor("bqc", [128, PCH], F32, kind="ExternalInput")
    bkc = nc.dram_tensor("bkc", [128, PCH], F32, kind="ExternalInput")
    cbc = nc.dram_tensor("cbc", [128, D], F32, kind="ExternalInput")
    out = nc.dram_tensor("out", [QT, D], MM_DT, kind="ExternalOutput")

    # xvT viewed as [128, PCH, S] so one DMA fetches a [128, PCH*128]
    # column-block (all 8 d-chunks of one key-token chunk).
    xvT3 = xvT.rearrange("(c p) s -> p c s", p=128)

    dbg = {}
    if DEBUG:
        dbg["kt"] = nc.dram_tensor("dbg_kt", [PCH, 128, S], MM_DT, kind="ExternalOutput")
        dbg["qt"] = nc.dram_tensor("dbg_qt", [PCH, 128, QT], MM_DT, kind="ExternalOutput")
        dbg["v"] = nc.dram_tensor("dbg_v", [KCH, 128, VPAD], MM_DT, kind="ExternalOutput")
        dbg["ot"] = nc.dram_tensor("dbg_ot", [PCH, 128, QT], MM_DT, kind="ExternalOutput")
        dbg["pt"] = nc.dram_tensor("dbg_pt", [2, 128, 2 * QH], MM_DT, kind="ExternalOutput")
        dbg["rc"] = nc.dram_tensor("dbg_rc", [2, 1, QH], F32, kind="ExternalOutput")

    from contextlib import ExitStack
    with ExitStack() as stack:
        pool = lambda name, bufs, **kw: stack.enter_context(
            tc.tile_pool(name=name, bufs=bufs, **kw))
        xkp = pool("xkp", PCH)            # xk chunks, live to hp6
        wkp = pool("wkp", PCH)
        xqp = pool("xqp", PCH)
        wqp = pool("wqp", PCH)
        xvp = pool("xvp", 2)              # [128, 1024] column-blocks
        wp = pool("wp", PCH)              # Wv then Wo
        ktp = pool("ktp", 3)              # K^T rolling
        qtp = pool("qtp", 3)              # Q^T rolling
        vp = pool("vp", KCH)              # V (ones-augmented) resident
        otp = pool("otp", PCH)            # O^T resident
        misc = pool("misc", 1)
        ptp = pool("ptp", 5)              # P^T staging
        rcp = pool("rcp", 2)
        bbp = pool("bbp", 2)
        outp = pool("outp", 3)
        bq_t = misc.tile([128, PCH], F32, name="bq_t")
        nc.sync.dma_start(out=bq_t[:, :], in_=bqc[:, :])
        bk_t = misc.tile([128, PCH], F32, name="bk_t")
        nc.sync.dma_start(out=bk_t[:, :], in_=bkc[:, :])
        cb_t = misc.tile([128, D], F32, name="cb_t")
        nc.sync.dma_start(out=cb_t[:, :], in_=cbc[:, :])
        ones_f = misc.tile([1, DH], F32, name="ones_f")
        nc.vector.memset(ones_f[:, :], 1.0)
        ones_t = misc.tile([1, DH], mybir.dt.float32r, name="ones_t")
        nc.vector.tensor_copy(ones_t[:, :], ones_f[:, :])

        # ---- prefix DMAs in three sequenced batches (K inputs get the HBM
        # bandwidth first; Q's batch starts only once K's data has landed,
        # then V's) so the first projections aren't starved by round-robin
        # across all queues.
        from concourse.tile_rust import add_dep_helper
        # DMA order: the K0-nb0/nb1 + Q0 critical set (xk halves 0, wk0, xq,
        # wq0) drains first; then xk halves 1 (K0-nb2/3, woven into pass 0),
        # then V's inputs, then the remaining weight chunks for the weave.
        xk_t = [xkp.tile([128, S], MM_DT, name=f"xk{i}", tag="xk")
                for i in range(PCH)]
        for i in range(PCH):
            nc.sync.dma_start(out=xk_t[i][:, 0:1024],
                              in_=xkT[i * 128:(i + 1) * 128, 0:1024])
        wk_t = [wkp.tile([128, D], MM_DT, name=f"wk{i}", tag="wk")
                for i in range(PCH)]
        nc.sync.dma_start(out=wk_t[0][:, :], in_=Wk[0:128, :])
        xq_t = []
        for i in range(PCH):
            xq = xqp.tile([128, QT], MM_DT, name=f"xq{i}", tag="xq")
            nc.sync.dma_start(out=xq[:, :], in_=xqT[i * 128:(i + 1) * 128, :])
            xq_t.append(xq)
        wq_t = [wqp.tile([128, D], MM_DT, name=f"wq{i}", tag="wq")
                for i in range(PCH)]
        nc.sync.dma_start(out=wq_t[0][:, :], in_=Wq[0:128, :])
        for i in range(PCH):
            nc.sync.dma_start(out=xk_t[i][:, 1024:2048],
                              in_=xkT[i * 128:(i + 1) * 128, 1024:2048])
        wv_t = []
        for i in range(PCH):
            wv = wp.tile([128, D], MM_DT, name=f"wv{i}", tag="w")
            nc.sync.dma_start(out=wv[:, :], in_=Wv[i * 128:(i + 1) * 128, :])
            wv_t.append(wv)
        xv_c = [xvp.tile([128, PCH * 128], MM_DT, name=f"xv{t}", tag="xv")
                for t in range(KCH)]

        def dma_xv(t):
            nc.sync.dma_start(
                out=xv_c[t][:, :].rearrange("p (c s) -> p c s", c=PCH),
                in_=xvT3[:, :, t * 128:(t + 1) * 128],
            )

        for t in range(KCH):
            dma_xv(t)
        for i in range(1, PCH):
            nc.sync.dma_start(out=wk_t[i][:, :], in_=Wk[i * 128:(i + 1) * 128, :])
            nc.sync.dma_start(out=wq_t[i][:, :], in_=Wq[i * 128:(i + 1) * 128, :])

        psum_stack = stack.enter_context(ExitStack())
        ppool = lambda name, bufs: psum_stack.enter_context(
            tc.tile_pool(name=name, bufs=bufs, space="PSUM"))
        pssp = ppool("pss", 2)
        pop = ppool("pop", 2)
        scrp = ppool("scr", 2)
        if True:
            kt_t, qt_t = {}, {}

            def proj_steps(which, m):
                """Emit-closures for one K^T/Q^T projection, one matmul per
                step so they can be woven between attention iterations."""
                if which == "k":
                    W, X, bias, nnb, dst = wk_t, xk_t, bk_t, S // 512, \
                        ktp.tile([128, S], MM_DT, name=f"kt{m}", tag="kt")
                    kt_t[m] = dst
                else:
                    W, X, bias, nnb, dst = wq_t, xq_t, bq_t, QT // 512, \
                        qtp.tile([128, QT], MM_DT, name=f"qt{m}", tag="qt")
                    qt_t[m] = dst
                steps = []
                for nb in range(nnb):
                    box = {}
                    def mk(nb, kk, box):
                        def step():
                            if kk == 0:
                                box["ps"] = scrp.tile(
                                    [128, 512], F32,
                                    name=f"ps{which}{m}_{nb}", tag="scr")
                            nc.tensor.matmul(
                                box["ps"][:, :],
                                lhsT=W[kk][:, m * 128:(m + 1) * 128],
                                rhs=X[kk][:, nb * 512:(nb + 1) * 512],
                                start=(kk == 0), stop=(kk == PCH - 1),
                                skip_group_check=True,
                            )
                            if kk == PCH - 1:
                                nc.vector.tensor_scalar(
                                    dst[:, nb * 512:(nb + 1) * 512],
                                    box["ps"][:, :],
                                    bias[:, m:m + 1], None, OP.add,
                                )
                                if DEBUG and nb == nnb - 1:
                                    nc.sync.dma_start(
                                        out=dbg["kt" if which == "k" else "qt"][m],
                                        in_=dst[:, :])
                        return step
                    for kk in range(PCH):
                        steps.append(mk(nb, kk, box))
                return steps

            def kproj(m):
                for s in proj_steps("k", m):
                    s()

            def qproj(m):
                for s in proj_steps("q", m):
                    s()

            v_t = [vp.tile([128, VPAD], MM_DT, name=f"v{t}", tag="v")
                   for t in range(KCH)]

            def vproj(t):
                oc = v_t[t][:, 0:H * VW].rearrange("p (h x) -> p h x", x=VW)
                nc.vector.memset(oc[:, :, DH:VW], 1.0)
                nc.vector.memset(v_t[t][:, H * VW:VPAD], 0.0)
                for db in range(D // 512):
                    ps = scrp.tile([128, 512], F32, name=f"psv{t}_{db}", tag="scr")
                    for kk in range(PCH):
                        nc.tensor.matmul(
                            ps[:, :],
                            lhsT=xv_c[t][:, kk * 128:(kk + 1) * 128],
                            rhs=wv_t[kk][:, db * 512:(db + 1) * 512],
                            start=(kk == 0), stop=(kk == PCH - 1),
                        )
                    dst = oc[:, db * 8:(db + 1) * 8, 0:DH]
                    src = ps[:, :].rearrange("p (h d) -> p h d", d=DH)
                    nc.vector.tensor_copy(dst, src)
                if DEBUG:
                    nc.sync.dma_start(out=dbg["v"][t], in_=v_t[t][:, :])

            # prefix: K0's first two token-blocks + all of Q0 (covers pass-0
            # scores t<8); K0-nb2/3 weave into pass 0 once xk's second halves
            # land. Pair 1's projections weave into pass 0's second q-half.
            k0_steps = proj_steps("k", 0)
            for s in k0_steps[0:16]:
                s()
            qproj(0)
            k0_rest = k0_steps[16:]

            ot_t = [otp.tile([128, QT], MM_DT, name=f"ot{i}", tag="ot")
                    for i in range(PCH)]

            def attn_v(hp, t, po, pt):
                # lhsT reads 128 cols (overlapping the next head's V block) so
                # the weight load takes the fast path; PSUM rows 65-127 get
                # garbage that is never read.
                for j in range(2):
                    h = 2 * hp + j
                    nc.tensor.matmul(
                        po[j][:, :],
                        lhsT=v_t[t][:, h * VW:h * VW + 128],
                        rhs=pt[:, j * QH:(j + 1) * QH],
                        start=(t == 0), stop=(t == KCH - 1),
                        skip_group_check=True,
                    )

            # Boundary work deferred into the following pass so it never
            # head-of-line blocks the PE stream: pending_tail holds the last
            # attnV + PSUM-evacuating copies (phase A), norm_b the reciprocal
            # -> broadcast -> multiply chain (phase B).
            norm_b = []
            pending_tail = []

            for hp in range(H // 2):
                # deferred projections for head-pair hp+1, woven one matmul
                # at a time into this pass's iteration stream so they fill
                # the PE slack while the exp stream paces the pass. Pass 0
                # weaves only in its second q-half (the weights arrive late
                # and a stalled weave matmul would block the whole stream).
                weave = []
                if hp <= H // 2 - 2:
                    weave = proj_steps("k", hp + 1) + proj_steps("q", hp + 1)
                wi = 0
                for qh in range(2):
                    po = [pop.tile([128, QH], F32, name=f"po{hp}_{qh}_{j}", tag="po")
                          for j in range(2)]
                    pt_prev = None
                    for t in range(KCH):
                        pss = pssp.tile([128, 2 * QH], F32,
                                        name=f"pss{hp}_{qh}_{t}", tag="pss")
                        for j in range(2):
                            lo, hi = j * 64, (j + 1) * 64
                            nc.tensor.matmul(
                                pss[:, j * QH:(j + 1) * QH],
                                lhsT=kt_t[hp][lo:hi, t * 128:(t + 1) * 128],
                                rhs=qt_t[hp][lo:hi, qh * QH:(qh + 1) * QH],
                                start=True, stop=True,
                            )
                        pt = ptp.tile([128, 2 * QH], MM_DT,
                                      name=f"pt{hp}_{qh}_{t}", tag="pt")
                        nc.scalar.activation(pt[:, :], pss[:, :], AF.Exp,
                                             scale=1.0 / 8.0)
                        if DEBUG and hp == 0 and qh == 0 and t < 2:
                            nc.sync.dma_start(out=dbg["pt"][t], in_=pt[:, :])
                        if t == 0 and pending_tail:
                            for fn in pending_tail:
                                fn()
                            pending_tail = []
                        if t == 2 and norm_b:
                            for fn in norm_b:
                                fn()
                            norm_b = []
                        if hp == 0 and qh == 0:
                            if 5 <= t <= 8:
                                for s in k0_rest[(t - 5) * 4:(t - 4) * 4]:
                                    s()
                            vproj(t)
                        if hp > 0 or qh == 1:
                            # finish the weave ~4 iterations before pass end
                            # so kt[hp+1]'s final evacuation never gates the
                            # next pass's first scores.
                            done = (qh * KCH + t) if hp else t
                            it_left = (2 * KCH if hp else KCH) - 4 - done
                            n_pop = (len(weave) - wi + it_left - 1) // it_left \
                                if it_left > 0 else len(weave) - wi
                            for _ in range(n_pop):
                                if wi < len(weave):
                                    weave[wi]()
                                    wi += 1
                        if pt_prev is not None:
                            attn_v(hp, t - 1, po, pt_prev)
                        pt_prev = pt

                    def mk_tail_a(hp, qh, po, pt_last):
                        def tail():
                            attn_v(hp, KCH - 1, po, pt_last)
                            # phase A: copy sums row + O rows off PSUM so the
                            # po accumulators recycle.
                            sums_j, ou_j = [], []
                            for j in range(2):
                                sums = rcp.tile([1, QH], F32,
                                                name=f"sm{hp}_{qh}_{j}", tag="sm")
                                nc.vector.tensor_copy(sums[:, :], po[j][64:65, :])
                                ou = bbp.tile([64, QH], F32,
                                              name=f"ou{hp}_{qh}_{j}", tag="ou")
                                nc.vector.tensor_copy(ou[:, :], po[j][0:64, :])
                                sums_j.append(sums)
                                ou_j.append(ou)
                            norm_b.append(mk_norm_b(hp, qh, sums_j, ou_j))
                        return tail

                    pending_tail.append(mk_tail_a(hp, qh, po, pt_prev))

                    def mk_norm_b(hp, qh, sums_j, ou_j):
                        def norm():
                            for j in range(2):
                                recip_f = rcp.tile([1, QH], F32,
                                                   name=f"rf{hp}_{qh}_{j}", tag="rf")
                                nc.vector.reciprocal_approx_fast(
                                    recip_f[:, :], sums_j[j][:, :])
                                recip = rcp.tile([1, QH], mybir.dt.float32r,
                                                 name=f"rc{hp}_{qh}_{j}", tag="rc")
                                nc.vector.tensor_copy(recip[:, :], recip_f[:, :])
                                if DEBUG and hp == 0 and qh == 0:
                                    nc.sync.dma_start(out=dbg["rc"][j],
                                                      in_=recip_f[:, :])
                                psb = scrp.tile([128, QH], F32,
                                                name=f"psb{hp}_{qh}_{j}", tag="scr")
                                nc.tensor.matmul(
                                    psb[0:64, :], lhsT=ones_t[:, :],
                                    rhs=recip[:, :], start=True, stop=True,
                                )
                                nc.vector.tensor_tensor(
                                    ot_t[hp][j * 64:(j + 1) * 64,
                                             qh * QH:(qh + 1) * QH],
                                    ou_j[j][:, :], psb[0:64, :], OP.mult,
                                )
                        return norm

                while wi < len(weave):
                    weave[wi]()
                    wi += 1

            for fn in pending_tail:
                fn()
            for fn in norm_b:
                fn()

        # ---- out-proj tail: out = O^T.T @ Wo + (bv@Wo + bo) ---------------
        psum_stack.close()
        wo_t = []
        for i in range(PCH):
            wo = wp.tile([128, D], MM_DT, name=f"wo{i}", tag="w")
            nc.sync.dma_start(out=wo[:, :], in_=Wo[i * 128:(i + 1) * 128, :])
            wo_t.append(wo)
        with tc.tile_pool(name="ps3", bufs=3, space="PSUM") as ps3:
            for qc in range(QT // 128):
                for db in range(D // 512):
                    ps = ps3.tile([128, 512], F32, name=f"pso{qc}_{db}", tag="ps3")
                    for vc in range(PCH):
                        nc.tensor.matmul(
                            ps[:, :],
                            lhsT=ot_t[vc][:, qc * 128:(qc + 1) * 128],
                            rhs=wo_t[vc][:, db * 512:(db + 1) * 512],
                            start=(vc == 0), stop=(vc == PCH - 1),
                        )
                    osb = outp.tile([128, 512], MM_DT, name=f"osb{qc}_{db}", tag="osb")
                    nc.vector.tensor_tensor(osb[:, :], ps[:, :],
                                            cb_t[:, db * 512:(db + 1) * 512], OP.add)
                    nc.sync.dma_start(
                        out=out[qc * 128:(qc + 1) * 128, db * 512:(db + 1) * 512],
                        in_=osb[:, :],
                    )


_NC_CACHE = None


def build_nc():
    global _NC_CACHE
    if _NC_CACHE is None:
        nc = bacc.Bacc("TRN2", target_bir_lowering=False, debug=False,
                       num_devices=N_CORES)
        with TileContext(nc) as tc:
            _emit(nc, tc)
        nc.compile()
        _NC_CACHE = nc
    return _NC_CACHE


def make_in_maps(query, key, value, Wq, bq, Wk, bk, Wv, bv, Wo, bo):
    c = (bv.astype(np.float32) @ Wo.astype(np.float32)) + bo.astype(np.float32)

    def q8(x, scale):
        return np.clip(np.asarray(x, np.float32) * scale, -240.0, 240.0).astype(NP_FP8)

    shared = {
        "Wq": np.ascontiguousarray(Wq, dtype=NP_MM),
        "Wk": np.ascontiguousarray(Wk, dtype=NP_MM),
        "Wv": np.ascontiguousarray(Wv, dtype=NP_MM),
        "Wo": np.ascontiguousarray(Wo, dtype=NP_MM),
        "bqc": np.ascontiguousarray(bq.reshape(PCH, 128).T, dtype=np.float32),
        "bkc": np.ascontiguousarray(bk.reshape(PCH, 128).T, dtype=np.float32),
        "cbc": np.ascontiguousarray(np.broadcast_to(c, (128, D)), dtype=np.float32),
    }
    in_maps = []
    for core in range(N_CORES):
        b, qh = core // 2, core % 2
        in_maps.append(dict(
            shared,
            xqT=np.ascontiguousarray(query[b, qh * QT:(qh + 1) * QT, :].T, dtype=NP_MM),
            xkT=np.ascontiguousarray(key[b].T, dtype=NP_MM),
            xvT=np.ascontiguousarray(value[b].T, dtype=NP_MM),
        ))
    return in_maps


def run(in_maps, trace=False):
    nc = build_nc()
    return run_bass_kernel_spmd(nc, in_maps, list(range(N_CORES)), trace=trace)


def kernel(query, key, value, mask, Wq, bq, Wk, bk, Wv, bv, Wo, bo):
    query = np.asarray(query, dtype=np.float32)
    key = np.asarray(key, dtype=np.float32)
    value = np.asarray(value, dtype=np.float32)
    # mask is all-ones by construction (spec fill: ones) — no-op in the math.
    in_maps = make_in_maps(query, key, value,
                           np.asarray(Wq), np.asarray(bq), np.asarray(Wk),
                           np.asarray(bk), np.asarray(Wv), np.asarray(bv),
                           np.asarray(Wo), np.asarray(bo))
    res = run(in_maps, trace=False)
    out = np.empty((B, S, D), np.float32)
    for core in range(N_CORES):
        b, qh = core // 2, core % 2
        out[b, qh * QT:(qh + 1) * QT, :] = np.asarray(
            res.results[core]["out"], dtype=np.float32)
    return out
